# revision 4
# baseline (speedup 1.0000x reference)
"""Multi-head causal attention (GPT-2 style) on 8 TRN2 NeuronCores.

Sharding: core i handles batch i//2 and head-group i%2 (8 of 16 heads,
i.e. a 512-wide slice of the QKV projections and of the Wp rows).  Each
core computes a partial output-projection for its batch; partials from
the two cores of a batch are summed on the host (cheap 4MB adds), along
with the exactly-factored bias terms:
  - bq is added to Q on-device (affects scores per key-column),
  - bk is dropped (adds a per-query constant to scores: softmax-invariant),
  - bv and bp commute through attention (rows of attn sum to 1):
    y += bv @ Wp + bp, applied on host.

On-chip layout (per core), T=1024, C=1024, DH=64:
  xT   [C, T]   x transposed (host-side transpose)         -> rhs / lhsT
  Q^T  [512, T] = (Wq_s*s)^T x^T  (s=1/8 folded into Wq)   -> scores rhs
  K^T  [512, T]                                            -> scores lhsT
  V    [T, 8, 65] natural layout + ones column             -> ctx lhsT
  S^T  [k-tile 128, q-chunk 512] scores transposed: softmax denominator
       comes out of the ctx matmul via the ones column of V; causal mask
       applied as an elementwise multiply on exp(S^T) diagonal blocks.
  ctx^T[512, T] normalized context                         -> yproj lhsT
All matmuls run in float32r (1 cycle/row on the PE at N=512; ~1e-4
relative accuracy), accumulation in fp32 PSUM.
"""
import numpy as np

import concourse.bacc as bacc
import concourse.mybir as mybir
import concourse.tile as tile
from concourse.bass_utils import run_bass_kernel_spmd

B, T, C, H, DH = 4, 1024, 1024, 16, 64
P = 128
CS = 512            # per-core head-slice width (8 heads * 64)
F32 = mybir.dt.float32
F32R = mybir.dt.float32r
BF16 = mybir.dt.bfloat16
MM_DTYPE = BF16     # matmul operand dtype: F32R (accurate) or BF16 (fast)
AF = mybir.ActivationFunctionType
SPLIT_EXP = True
N_CORES = 8


def build_nc(loop_n=None, mm_dtype=None, phase='full', proj_bufs=2, copy_eng='dve'):
    MMD = mm_dtype or MM_DTYPE
    nc = bacc.Bacc("TRN2", target_bir_lowering=False, debug=False,
                   num_devices=N_CORES)
    xT = nc.dram_tensor("xT", [C, T], MMD, kind="ExternalInput")
    wq = nc.dram_tensor("wq", [C, CS], MMD, kind="ExternalInput")
    wk = nc.dram_tensor("wk", [C, CS], MMD, kind="ExternalInput")
    wv = nc.dram_tensor("wv", [C, CS], MMD, kind="ExternalInput")
    wp = nc.dram_tensor("wp", [CS, C], MMD, kind="ExternalInput")
    bq = nc.dram_tensor("bq", [P, 4], F32, kind="ExternalInput")
    mask = nc.dram_tensor("mask", [P, P], F32, kind="ExternalInput")
    y = nc.dram_tensor("y", [T, C], F32, kind="ExternalOutput")
    dbg = (nc.dram_tensor("dbg", [P, 3, 4224], MMD, kind="ExternalOutput")
           if phase != 'full' else None)

    with tile.TileContext(nc) as tc:
        with (
            tc.tile_pool(name="big", bufs=1) as big,
            tc.tile_pool(name="es_pool", bufs=3) as es_pool,
            tc.tile_pool(name="y_pool", bufs=3) as y_pool,
            tc.tile_pool(name="small", bufs=2) as small,
            tc.tile_pool(name="proj_ps", bufs=proj_bufs, space="PSUM") as proj_ps,
            tc.tile_pool(name="sc_ps", bufs=2, space="PSUM") as sc_ps,
            tc.tile_pool(name="ctx_ps", bufs=1, space="PSUM") as ctx_ps,
        ):
            from contextlib import ExitStack
            _ls = ExitStack()
            if loop_n:
                _ls.enter_context(tc.For_i(0, loop_n, 1))
            xT_sb = big.tile([P, 8, T], MMD)
            wq_sb = big.tile([P, 8, CS], MMD)
            wk_sb = big.tile([P, 8, CS], MMD)
            wv_sb = big.tile([P, 8, CS], MMD)
            wp_sb = big.tile([P, 4, C], MMD)
            bq_sb = big.tile([P, 4], F32)
            mask_sb = big.tile([P, P], F32)
            qT_sb = big.tile([P, 4, 2, 512], MMD)
            kT_sb = big.tile([P, 4, 2, 512], MMD)
            v_sb = big.tile([P, 8, 8, 65], MMD)
            ctxT_sb = big.tile([P, 4, T], MMD)

            nc.sync.dma_start(out=bq_sb, in_=bq.ap())
            nc.sync.dma_start(out=mask_sb, in_=mask.ap())
            xT_r = xT.ap().rearrange("(c p) t -> p c t", p=P)
            wq_r = wq.ap().rearrange("(c p) n -> p c n", p=P)
            wk_r = wk.ap().rearrange("(c p) n -> p c n", p=P)
            wv_r = wv.ap().rearrange("(c p) n -> p c n", p=P)
            for c in range(8):
                nc.sync.dma_start(out=xT_sb[:, c, :], in_=xT_r[:, c, :])
                nc.sync.dma_start(out=wv_sb[:, c, :], in_=wv_r[:, c, :])
                nc.sync.dma_start(out=wq_sb[:, c, :], in_=wq_r[:, c, :])
                nc.sync.dma_start(out=wk_sb[:, c, :], in_=wk_r[:, c, :])
            wp_r = wp.ap().rearrange("(k p) n -> p k n", p=P)
            for kc in range(4):
                nc.sync.dma_start(out=wp_sb[:, kc, :], in_=wp_r[:, kc, :])

            # ---- V natural [T, 512] + ones column per head ----
            # Engine memset, NOT a scattered DMA: the stride-65 single-element
            # DMA write is not reliably ordered before the ctx matmul's first
            # read on HW (first-run garbage denominators).
            nc.vector.memset(v_sb[:, :, :, 64], 1.0)
            for tt in range(8):
                ps = proj_ps.tile([P, 512], F32, tag="proj")
                for c in range(8):
                    nc.tensor.matmul(
                        ps, xT_sb[:, c, tt * P:(tt + 1) * P], wv_sb[:, c, :],
                        start=(c == 0), stop=(c == 7))
                if copy_eng == 'dve':
                    nc.vector.tensor_copy(
                        v_sb[:, tt, :, 0:64],
                        ps.rearrange("p (h d) -> p h d", h=8))
                else:
                    nc.scalar.copy(
                        v_sb[:, tt, :, 0:64],
                        ps.rearrange("p (h d) -> p h d", h=8))

            def qk_proj(mc):
                for wsb, outsb, is_q in ((wq_sb, qT_sb, True), (wk_sb, kT_sb, False)):
                    for tc2 in range(2):
                        ps = proj_ps.tile([P, 512], F32, tag="proj", name="qkps")
                        for c in range(8):
                            nc.tensor.matmul(
                                ps, wsb[:, c, mc * P:(mc + 1) * P],
                                xT_sb[:, c, tc2 * 512:(tc2 + 1) * 512],
                                start=(c == 0), stop=(c == 7))
                        dst = outsb[:, mc, tc2, :]
                        if is_q:
                            nc.vector.tensor_add(
                                dst, ps,
                                bq_sb[:, mc:mc + 1].broadcast_to([P, 512]))
                        elif copy_eng == 'dve':
                            nc.vector.tensor_copy(dst, ps)
                        else:
                            nc.scalar.copy(dst, ps)

            def norm_write(h, qc, cps):
                hp = (h % 2) * 64
                mc = h // 2
                recr = small.tile([1, 512], F32, tag="recr", name="recr")
                nc.vector.reciprocal(recr, cps[64:65, :])
                recb = small.tile([64, 512], F32, tag="recb", name="recb")
                nc.gpsimd.partition_broadcast(recb, recr)
                nc.vector.tensor_mul(
                    ctxT_sb[hp:hp + 64, mc, qc * 512:(qc + 1) * 512],
                    cps[0:64, :], recb)

            def attention(h):
                # merged q-chunks: one exp over the contiguous valid q-range
                hp = (h % 2) * 64
                mc = h // 2
                cps0 = ctx_ps.tile([65, 512], F32, tag="ctx0", name="cps0")
                cps1 = ctx_ps.tile([65, 512], F32, tag="ctx1", name="cps1")
                for kt in range(8):
                    lhsT = kT_sb[hp:hp + 64, mc, kt // 4,
                                 (kt % 4) * P:(kt % 4 + 1) * P]
                    sps = sc_ps.tile([P, 2, 512], F32, tag="sc", name="sps")
                    es = es_pool.tile([P, 2, 512], MMD, tag="es", name="es")
                    if kt < 4:
                        r0 = kt * P
                        nc.tensor.matmul(sps[:, 0, r0:], lhsT,
                                         qT_sb[hp:hp + 64, mc, 0, r0:],
                                         start=True, stop=True)
                        nc.tensor.matmul(sps[:, 1, :], lhsT,
                                         qT_sb[hp:hp + 64, mc, 1, :],
                                         start=True, stop=True)
                        if SPLIT_EXP:
                            nc.scalar.activation(es[:, 0, r0:], sps[:, 0, r0:],
                                                 AF.Exp)
                            nc.scalar.activation(es[:, 1, :], sps[:, 1, :],
                                                 AF.Exp)
                        else:
                            nc.scalar.activation(
                                es.rearrange("p a b -> p (a b)")[:, r0:],
                                sps.rearrange("p a b -> p (a b)")[:, r0:], AF.Exp)
                        nc.vector.tensor_mul(es[:, 0, r0:r0 + P],
                                             es[:, 0, r0:r0 + P], mask_sb)
                        nc.tensor.matmul(cps0[:, r0:], v_sb[:, kt, h, :],
                                         es[:, 0, r0:],
                                         start=(kt == 0), stop=(kt == 3))
                        nc.tensor.matmul(cps1, v_sb[:, kt, h, :], es[:, 1, :],
                                         start=(kt == 0), stop=(kt == 7))
                    else:
                        r0 = (kt - 4) * P
                        nc.tensor.matmul(sps[:, 0, r0:], lhsT,
                                         qT_sb[hp:hp + 64, mc, 1, r0:],
                                         start=True, stop=True)
                        nc.scalar.activation(es[:, 0, r0:], sps[:, 0, r0:], AF.Exp)
                        nc.vector.tensor_mul(es[:, 0, r0:r0 + P],
                                             es[:, 0, r0:r0 + P], mask_sb)
                        nc.tensor.matmul(cps1[:, r0:], v_sb[:, kt, h, :],
                                         es[:, 0, r0:],
                                         start=False, stop=(kt == 7))
                    if kt == 3:
                        norm_write(h, 0, cps0)
                norm_write(h, 1, cps1)

            def yproj(tt_range):
                for tt in tt_range:
                    for nk in range(2):
                        ps = proj_ps.tile([P, 512], F32, tag="proj", name="yps")
                        for kc in range(4):
                            nc.tensor.matmul(
                                ps, ctxT_sb[:, kc, tt * P:(tt + 1) * P],
                                wp_sb[:, kc, nk * 512:(nk + 1) * 512],
                                start=(kc == 0), stop=(kc == 3))
                        ysb = y_pool.tile([P, 512], F32, tag="y", name="ysb")
                        nc.vector.tensor_copy(ysb, ps)
                        nc.sync.dma_start(
                            out=y.ap()[tt * P:(tt + 1) * P,
                                       nk * 512:(nk + 1) * 512],
                            in_=ysb)

            if phase == 'dma':
                for di, sb_t in enumerate((xT_sb, wq_sb, wk_sb, wv_sb, wp_sb)):
                    nch = sb_t.shape[1]
                    nc.sync.dma_start(
                        out=dbg.ap()[:, 0, di * 64:di * 64 + nch * 8],
                        in_=sb_t[:, :, :8])
            elif phase == 'proj':
                for mc in range(4):
                    qk_proj(mc)
                nc.sync.dma_start(out=dbg.ap()[:, 0, :4096],
                                  in_=qT_sb.rearrange("p a b c -> p (a b c)"))
                nc.sync.dma_start(out=dbg.ap()[:, 1, :4096],
                                  in_=kT_sb.rearrange("p a b c -> p (a b c)"))
                nc.sync.dma_start(out=dbg.ap()[:, 2, :4160],
                                  in_=v_sb.rearrange("p a b c -> p (a b c)"))
            elif phase == 'attn':
                for mc in range(4):
                    qk_proj(mc)
                    attention(2 * mc)
                    attention(2 * mc + 1)
                nc.sync.dma_start(out=dbg.ap()[:, 0, :4096],
                                  in_=ctxT_sb.rearrange("p a b -> p (a b)"))
            else:
                for mc in range(4):
                    qk_proj(mc)
                    attention(2 * mc)
                    attention(2 * mc + 1)
                    if mc == 3:
                        yproj(range(8))
            _ls.close()
    nc.compile()
    return nc


_NC = None


def _get_nc():
    global _NC
    if _NC is None:
        _NC = build_nc()
    return _NC


def make_in_maps(x, Wq, bq, Wk, Wv, Wp, mm_dtype=None):
    """Per-core input dicts."""
    import ml_dtypes
    MMD = mm_dtype or MM_DTYPE
    cvt = ((lambda a: np.ascontiguousarray(a).astype(ml_dtypes.bfloat16))
           if MMD == BF16 else np.ascontiguousarray)
    masks = (np.arange(P)[None, :] >= np.arange(P)[:, None]).astype(np.float32)
    in_maps = []
    for core in range(N_CORES):
        b = core // 2
        g = core % 2
        cs = slice(g * CS, (g + 1) * CS)
        in_maps.append(dict(
            xT=cvt(x[b].T),
            wq=cvt(Wq[:, cs] * np.float32(0.125)),
            wk=cvt(Wk[:, cs]),
            wv=cvt(Wv[:, cs]),
            wp=cvt(Wp[cs, :]),
            bq=np.ascontiguousarray((bq[cs] * np.float32(0.125))
                                    .reshape(4, P).T),
            mask=masks,
        ))
    return in_maps


def combine(parts, Wq, bv, Wp, bp):
    """parts: list of 8 per-core partial y arrays -> full [B, T, C] output."""
    out = np.stack([parts[2 * b] + parts[2 * b + 1] for b in range(B)])
    out += (bv @ Wp + bp)[None, None, :]
    return out.astype(np.float32)


def kernel(**inputs):
    x = np.asarray(inputs["x"], np.float32)
    Wq = np.asarray(inputs["Wq"], np.float32)
    bq = np.asarray(inputs["bq"], np.float32)
    Wk = np.asarray(inputs["Wk"], np.float32)
    Wv = np.asarray(inputs["Wv"], np.float32)
    Wp = np.asarray(inputs["Wp"], np.float32)
    bv = np.asarray(inputs["bv"], np.float32)
    bp = np.asarray(inputs["bp"], np.float32)
    # bk intentionally unused: it shifts every score of a query row by the
    # same amount, which softmax cancels exactly.

    nc = _get_nc()
    in_maps = make_in_maps(x, Wq, bq, Wk, Wv, Wp)
    res = run_bass_kernel_spmd(nc, in_maps, core_ids=list(range(N_CORES)))
    parts = [res.results[c]["y"] for c in range(N_CORES)]
    return combine(parts, Wq, bv, Wp, bp)



# revision 19
# speedup vs baseline: 4.3840x; 4.3840x over previous
"""Multi-head causal attention (GPT-2 style) on 8 TRN2 NeuronCores.

Sharding: core i handles batch i//2 and head-group i%2 (8 of 16 heads,
i.e. a 512-wide slice of the QKV projections and of the Wp rows).  Each
core computes a partial output-projection for its batch; partials from
the two cores of a batch are summed on the host (cheap 4MB adds), along
with the exactly-factored bias terms:
  - bq is added to Q on-device (affects scores per key-column),
  - bk is dropped (adds a per-query constant to scores: softmax-invariant),
  - bv and bp commute through attention (rows of attn sum to 1):
    y += bv @ Wp + bp, applied on host.

On-chip layout (per core), T=1024, C=1024, DH=64:
  xT   [C, T]   x transposed (host-side transpose)         -> rhs / lhsT
  Q^T  [512, T] = (Wq_s*s)^T x^T  (s=1/8 folded into Wq)   -> scores rhs
  K^T  [512, T]                                            -> scores lhsT
  V    [T, 8, 65] natural layout + ones column             -> ctx lhsT
  S^T  [k-tile 128, q-chunk 512] scores transposed; the softmax
       denominator comes out of the ctx matmul via the ones column of V.
  ctx^T[512, T] normalized context                         -> yproj lhsT
Causal mask: applied IN PSUM by an accumulating identity @ (-1e4*mask)
matmul on the diagonal blocks, so exp(scores) needs no separate
elementwise mask step (exp(-1e4) underflows to exactly 0).
Input DMAs are split across both HWDGE queues (SP + Activation).
"""
import numpy as np

import concourse.bacc as bacc
import concourse.mybir as mybir
import concourse.tile as tile
from concourse.bass_utils import run_bass_kernel_spmd

B, T, C, H, DH = 4, 1024, 1024, 16, 64
P = 128
CS = 512            # per-core head-slice width (8 heads * 64)
F32 = mybir.dt.float32
F32R = mybir.dt.float32r
BF16 = mybir.dt.bfloat16
MM_DTYPE = BF16     # matmul operand dtype
AF = mybir.ActivationFunctionType
SPLIT_EXP = False
VPAD = 96           # padded V row stride (elements); 192B-aligned lhsT rows
N_CORES = 8


def build_nc(loop_n=None, mm_dtype=None, phase='full', split_exp=None):
    MMD = mm_dtype or MM_DTYPE
    sexp = SPLIT_EXP if split_exp is None else split_exp
    nc = bacc.Bacc("TRN2", target_bir_lowering=False, debug=False,
                   num_devices=N_CORES)
    xT = nc.dram_tensor("xT", [C, T], MMD, kind="ExternalInput")
    wq = nc.dram_tensor("wq", [C, CS], MMD, kind="ExternalInput")
    wk = nc.dram_tensor("wk", [C, CS], MMD, kind="ExternalInput")
    wv = nc.dram_tensor("wv", [C, CS], MMD, kind="ExternalInput")
    wp = nc.dram_tensor("wp", [CS, C], MMD, kind="ExternalInput")
    bq = nc.dram_tensor("bq", [P, 4], F32, kind="ExternalInput")
    maskneg = nc.dram_tensor("maskneg", [P, P], MMD, kind="ExternalInput")
    ident = nc.dram_tensor("ident", [P, P], MMD, kind="ExternalInput")
    y = nc.dram_tensor("y", [T, C], BF16, kind="ExternalOutput")
    dbg = (nc.dram_tensor("dbg", [P, 3, 4224], MMD, kind="ExternalOutput")
           if phase != 'full' else None)

    with tile.TileContext(nc) as tc:
        with (
            tc.tile_pool(name="big", bufs=1) as big,
            tc.tile_pool(name="es_pool", bufs=4) as es_pool,
            tc.tile_pool(name="y_pool", bufs=3) as y_pool,
            tc.tile_pool(name="small", bufs=2) as small,
            tc.tile_pool(name="proj_ps", bufs=2, space="PSUM") as proj_ps,
            tc.tile_pool(name="sc_ps", bufs=2, space="PSUM") as sc_ps,
            tc.tile_pool(name="ctx_ps", bufs=1, space="PSUM") as ctx_ps,
        ):
            from contextlib import ExitStack
            _ls = ExitStack()
            if loop_n:
                _ls.enter_context(tc.For_i(0, loop_n, 1))
            xT_sb = big.tile([P, 8, T], MMD)
            wq_sb = big.tile([P, 8, CS], MMD)
            wk_sb = big.tile([P, 8, CS], MMD)
            wv_sb = big.tile([P, 8, CS], MMD)
            wp_sb = big.tile([P, 4, C], MMD)
            bq_sb = big.tile([P, 4], F32)
            mneg_sb = big.tile([P, P], MMD)
            id_sb = big.tile([P, P], MMD)
            qT_sb = big.tile([P, 4, 2, 512], MMD)
            kT_sb = big.tile([P, 4, 2, 512], MMD)
            v_sb = big.tile([P, 8, 8, VPAD], MMD)
            ctxT_sb = big.tile([P, 4, T], MMD)

            # Channel chunks use the "(p c)" split: partition p holds DRAM
            # rows p*8+c, i.e. 8 CONTIGUOUS rows -> one 8-16KB descriptor per
            # partition instead of eight 1-2KB ones.  The contraction is a
            # sum over all 1024 channels, and both matmul operands of every
            # chunk use the SAME permutation, so results are unchanged.
            nc.scalar.dma_start(out=bq_sb, in_=bq.ap())
            nc.scalar.dma_start(out=mneg_sb, in_=maskneg.ap())
            nc.scalar.dma_start(out=id_sb, in_=ident.ap())
            xT_r = xT.ap().rearrange("(p c) t -> p c t", p=P)
            wq_r = wq.ap().rearrange("(p c) n -> p c n", p=P)
            wk_r = wk.ap().rearrange("(p c) n -> p c n", p=P)
            wv_r = wv.ap().rearrange("(p c) n -> p c n", p=P)
            # xT+wv land first (V proj gates on them), one per HWDGE queue.
            nc.sync.dma_start(out=xT_sb, in_=xT_r)
            nc.scalar.dma_start(out=wv_sb, in_=wv_r)
            nc.sync.dma_start(out=wq_sb, in_=wq_r)
            nc.scalar.dma_start(out=wk_sb, in_=wk_r)
            wp_r = wp.ap().rearrange("(k p) n -> p k n", p=P)
            for kc in range(4):
                nc.sync.dma_start(out=wp_sb[:, kc, :], in_=wp_r[:, kc, :])

            # ---- V natural [T, 512] + ones column per head ----
            # Engine memset, NOT a scattered DMA: the stride-65 single-element
            # DMA write is not reliably ordered before the ctx matmul's first
            # read on HW (first-run garbage denominators).
            nc.vector.memset(v_sb[:, :, :, 64], 1.0)
            for tt in range(8):
                ps = proj_ps.tile([P, 512], F32, tag="proj")
                for c in range(8):
                    nc.tensor.matmul(
                        ps, xT_sb[:, c, tt * P:(tt + 1) * P], wv_sb[:, c, :],
                        start=(c == 0), stop=(c == 7))
                nc.vector.tensor_copy(
                    v_sb[:, tt, :, 0:64],
                    ps.rearrange("p (h d) -> p h d", h=8))

            _qk_done = set()

            def qk_proj_group(mc, gi):
                # gi: 0..3 = (wq,tc0),(wq,tc1),(wk,tc0),(wk,tc1)
                if (mc, gi) in _qk_done:
                    return
                _qk_done.add((mc, gi))
                wsb, outsb, is_q = ((wq_sb, qT_sb, True), (wk_sb, kT_sb, False))[gi // 2]
                tc2 = gi % 2
                ps = proj_ps.tile([P, 512], F32, tag="proj", name="qkps")
                for c in range(8):
                    nc.tensor.matmul(
                        ps, wsb[:, c, mc * P:(mc + 1) * P],
                        xT_sb[:, c, tc2 * 512:(tc2 + 1) * 512],
                        start=(c == 0), stop=(c == 7))
                dst = outsb[:, mc, tc2, :]
                if is_q:
                    nc.vector.tensor_scalar_add(dst, ps, bq_sb[:, mc:mc + 1])
                else:
                    nc.vector.tensor_copy(dst, ps)

            def qk_proj(mc):
                for gi in range(4):
                    qk_proj_group(mc, gi)

            def norm_write(h, qc, cps):
                hp = (h % 2) * 64
                mc = h // 2
                recr = small.tile([1, 512], F32, tag="recr", name="recr")
                nc.vector.reciprocal(recr, cps[64:65, :])
                recb = small.tile([64, 512], F32, tag="recb", name="recb")
                nc.gpsimd.partition_broadcast(recb, recr)
                nc.vector.tensor_mul(
                    ctxT_sb[hp:hp + 64, mc, qc * 512:(qc + 1) * 512],
                    cps[0:64, :], recb)

            _es_store = {}

            def emit_scores(h, kt):
                if (h, kt) in _es_store:
                    return
                hp = (h % 2) * 64
                mc = h // 2
                lhsT = kT_sb[hp:hp + 64, mc, kt // 4,
                             (kt % 4) * P:(kt % 4 + 1) * P]
                sps = sc_ps.tile([P, 2, 512], F32, tag="sc", name="sps")
                es = es_pool.tile([P, 2, 512], MMD, tag="es", name="es")
                _es_store[(h, kt)] = es
                r0 = (kt % 4) * P
                if kt < 4:
                    nc.tensor.matmul(sps[:, 0, r0:], lhsT,
                                     qT_sb[hp:hp + 64, mc, 0, r0:],
                                     start=True, stop=True)
                    nc.tensor.matmul(sps[:, 1, :], lhsT,
                                     qT_sb[hp:hp + 64, mc, 1, :],
                                     start=True, stop=True)
                else:
                    nc.tensor.matmul(sps[:, 0, r0:], lhsT,
                                     qT_sb[hp:hp + 64, mc, 1, r0:],
                                     start=True, stop=True)
                # causal mask: accumulate -1e4 above the diagonal, in PSUM
                nc.tensor.matmul(sps[:, 0, r0:r0 + P], id_sb, mneg_sb,
                                 start=False, stop=True,
                                 skip_group_check=True)
                if kt < 4:
                    if sexp:
                        nc.scalar.activation(es[:, 0, r0:], sps[:, 0, r0:],
                                             AF.Exp)
                        nc.scalar.activation(es[:, 1, :], sps[:, 1, :], AF.Exp)
                    else:
                        nc.scalar.activation(
                            es.rearrange("p a b -> p (a b)")[:, r0:],
                            sps.rearrange("p a b -> p (a b)")[:, r0:], AF.Exp)
                else:
                    nc.scalar.activation(es[:, 0, r0:], sps[:, 0, r0:], AF.Exp)

            def attention(h, pre_tail=None):
                # Software-pipelined over kt: ctx(kt-1) is emitted AFTER
                # scores(kt)+mask(kt), so the in-order PE stream never waits
                # for exp(kt) before it has scores(kt+1) to chew on.
                # pre_tail() emits independent PE work (next head's first
                # scores / next projection group) before the final ctx, which
                # is the one matmul that must wait for exp(7).
                hp = (h % 2) * 64
                cps0 = ctx_ps.tile([65, 512], F32, tag="ctx0", name="cps0")
                cps1 = ctx_ps.tile([65, 512], F32, tag="ctx1", name="cps1")

                def emit_ctx(kt):
                    es = _es_store.pop((h, kt))
                    r0 = (kt % 4) * P
                    v_l = v_sb[:, kt, h, 0:65]
                    if kt < 4:
                        nc.tensor.matmul(cps0[:, r0:], v_l, es[:, 0, r0:],
                                         start=(kt == 0), stop=(kt == 3))
                        nc.tensor.matmul(cps1, v_l, es[:, 1, :],
                                         start=(kt == 0), stop=(kt == 7))
                    else:
                        nc.tensor.matmul(cps1[:, r0:], v_l, es[:, 0, r0:],
                                         start=False, stop=(kt == 7))
                    if kt == 3:
                        norm_write(h, 0, cps0)

                emit_scores(h, 0)
                for kt in range(1, 8):
                    emit_scores(h, kt)
                    emit_ctx(kt - 1)
                if pre_tail is not None:
                    pre_tail()
                emit_ctx(7)
                norm_write(h, 1, cps1)

            def yproj(tt_range):
                for tt in tt_range:
                    ps = sc_ps.tile([P, 2, 512], F32, tag="sc", name="yps")
                    for kc in range(4):
                        lhsT = ctxT_sb[:, kc, tt * P:(tt + 1) * P]
                        nc.tensor.matmul(ps[:, 0, :], lhsT, wp_sb[:, kc, 0:512],
                                         start=(kc == 0), stop=(kc == 3))
                        nc.tensor.matmul(ps[:, 1, :], lhsT, wp_sb[:, kc, 512:],
                                         start=(kc == 0), stop=(kc == 3))
                    ysb = y_pool.tile([P, C], BF16, tag="y", name="ysb")
                    nc.vector.tensor_copy(ysb, ps.rearrange("p a b -> p (a b)"))
                    (nc.sync, nc.scalar)[tt % 2].dma_start(
                        out=y.ap()[tt * P:(tt + 1) * P, :], in_=ysb)

            if phase == 'empty':
                pass
            elif phase == 'dma':
                for di, sb_t in enumerate((xT_sb, wq_sb, wk_sb, wv_sb, wp_sb)):
                    nch = sb_t.shape[1]
                    nc.sync.dma_start(
                        out=dbg.ap()[:, 0, di * 64:di * 64 + nch * 8],
                        in_=sb_t[:, :, :8])
            elif phase == 'proj':
                for mc in range(4):
                    qk_proj(mc)
                nc.sync.dma_start(out=dbg.ap()[:, 0, :4096],
                                  in_=qT_sb.rearrange("p a b c -> p (a b c)"))
                nc.sync.dma_start(out=dbg.ap()[:, 1, :4096],
                                  in_=kT_sb.rearrange("p a b c -> p (a b c)"))
                nc.sync.dma_start(out=dbg.ap()[:, 2, :4160],
                                  in_=v_sb[:, :, :, 0:65]
                                  .rearrange("p a b c -> p (a b c)"))
            elif phase == 'attn':
                for mc in range(4):
                    qk_proj(mc)
                    attention(2 * mc,
                              pre_tail=lambda m=mc: emit_scores(2 * m + 1, 0))
                    attention(2 * mc + 1,
                              pre_tail=(lambda m=mc: qk_proj_group(m + 1, 0))
                              if mc < 3 else None)
                nc.sync.dma_start(out=dbg.ap()[:, 0, :4096],
                                  in_=ctxT_sb.rearrange("p a b -> p (a b)"))
            else:
                for mc in range(4):
                    qk_proj(mc)
                    attention(2 * mc,
                              pre_tail=lambda m=mc: emit_scores(2 * m + 1, 0))
                    attention(2 * mc + 1,
                              pre_tail=(lambda m=mc: qk_proj_group(m + 1, 0))
                              if mc < 3 else None)
                    if mc == 3:
                        yproj(range(8))
            _ls.close()
    nc.compile()
    return nc


_NC = None


def _get_nc():
    global _NC
    if _NC is None:
        _NC = build_nc()
    return _NC


def make_in_maps(x, Wq, bq, Wk, Wv, Wp, mm_dtype=None):
    """Per-core input dicts."""
    import ml_dtypes
    MMD = mm_dtype or MM_DTYPE
    cvt = ((lambda a: np.ascontiguousarray(a).astype(ml_dtypes.bfloat16))
           if MMD == BF16 else np.ascontiguousarray)
    valid = (np.arange(P)[None, :] >= np.arange(P)[:, None])
    maskneg = np.where(valid, 0.0, -10000.0).astype(np.float32)
    in_maps = []
    for core in range(N_CORES):
        b = core // 2
        g = core % 2
        cs = slice(g * CS, (g + 1) * CS)
        in_maps.append(dict(
            xT=cvt(x[b].T),
            wq=cvt(Wq[:, cs] * np.float32(0.125)),
            wk=cvt(Wk[:, cs]),
            wv=cvt(Wv[:, cs]),
            wp=cvt(Wp[cs, :]),
            bq=np.ascontiguousarray((bq[cs] * np.float32(0.125))
                                    .reshape(4, P).T),
            maskneg=cvt(maskneg),
            ident=cvt(np.eye(P, dtype=np.float32)),
        ))
    return in_maps


def combine(parts, Wq, bv, Wp, bp):
    """parts: list of 8 per-core partial y arrays -> full [B, T, C] output."""
    out = np.stack([np.asarray(parts[2 * b], np.float32)
                    + np.asarray(parts[2 * b + 1], np.float32)
                    for b in range(B)])
    out += (bv @ Wp + bp)[None, None, :]
    return out.astype(np.float32)


def kernel(**inputs):
    x = np.asarray(inputs["x"], np.float32)
    Wq = np.asarray(inputs["Wq"], np.float32)
    bq = np.asarray(inputs["bq"], np.float32)
    Wk = np.asarray(inputs["Wk"], np.float32)
    Wv = np.asarray(inputs["Wv"], np.float32)
    Wp = np.asarray(inputs["Wp"], np.float32)
    bv = np.asarray(inputs["bv"], np.float32)
    bp = np.asarray(inputs["bp"], np.float32)
    # bk intentionally unused: it shifts every score of a query row by the
    # same amount, which softmax cancels exactly.

    nc = _get_nc()
    in_maps = make_in_maps(x, Wq, bq, Wk, Wv, Wp)
    res = run_bass_kernel_spmd(nc, in_maps, core_ids=list(range(N_CORES)))
    parts = [res.results[c]["y"] for c in range(N_CORES)]
    return combine(parts, Wq, bv, Wp, bp)


# revision 20
# speedup vs baseline: 4.5868x; 1.0462x over previous
"""Multi-head causal attention (GPT-2 style) on 8 TRN2 NeuronCores.

Sharding: core i handles batch i//2 and head-group i%2 (8 of 16 heads,
i.e. a 512-wide slice of the QKV projections and of the Wp rows).  Each
core computes a partial output-projection for its batch; partials from
the two cores of a batch are summed on the host (cheap 4MB adds), along
with the exactly-factored bias terms:
  - bq is added to Q on-device (affects scores per key-column),
  - bk is dropped (adds a per-query constant to scores: softmax-invariant),
  - bv and bp commute through attention (rows of attn sum to 1):
    y += bv @ Wp + bp, applied on host.

On-chip layout (per core), T=1024, C=1024, DH=64:
  xT   [C, T]   x transposed (host-side transpose)         -> rhs / lhsT
  Q^T  [512, T] = (Wq_s*s)^T x^T  (s=1/8 folded into Wq)   -> scores rhs
  K^T  [512, T]                                            -> scores lhsT
  V    [T, 8, 65] natural layout + ones column             -> ctx lhsT
  S^T  [k-tile 128, q-chunk 512] scores transposed; the softmax
       denominator comes out of the ctx matmul via the ones column of V.
  ctx^T[512, T] normalized context                         -> yproj lhsT
Causal mask: applied IN PSUM by an accumulating identity @ (-1e4*mask)
matmul on the diagonal blocks, so exp(scores) needs no separate
elementwise mask step (exp(-1e4) underflows to exactly 0).
Input DMAs are split across both HWDGE queues (SP + Activation).
"""
import numpy as np

import concourse.bacc as bacc
import concourse.mybir as mybir
import concourse.tile as tile
from concourse.bass_utils import run_bass_kernel_spmd

B, T, C, H, DH = 4, 1024, 1024, 16, 64
P = 128
CS = 512            # per-core head-slice width (8 heads * 64)
F32 = mybir.dt.float32
F32R = mybir.dt.float32r
BF16 = mybir.dt.bfloat16
MM_DTYPE = BF16     # matmul operand dtype
AF = mybir.ActivationFunctionType
SPLIT_EXP = False
VPAD = 96           # padded V row stride (elements); 192B-aligned lhsT rows
N_CORES = 8


def build_nc(loop_n=None, mm_dtype=None, phase='full', split_exp=None):
    MMD = mm_dtype or MM_DTYPE
    sexp = SPLIT_EXP if split_exp is None else split_exp
    nc = bacc.Bacc("TRN2", target_bir_lowering=False, debug=False,
                   num_devices=N_CORES)
    xT = nc.dram_tensor("xT", [C, T], MMD, kind="ExternalInput")
    wq = nc.dram_tensor("wq", [C, CS], MMD, kind="ExternalInput")
    wk = nc.dram_tensor("wk", [C, CS], MMD, kind="ExternalInput")
    wv = nc.dram_tensor("wv", [C, CS], MMD, kind="ExternalInput")
    wp = nc.dram_tensor("wp", [CS, C], MMD, kind="ExternalInput")
    bq = nc.dram_tensor("bq", [P, 4], F32, kind="ExternalInput")
    maskneg = nc.dram_tensor("maskneg", [P, P], MMD, kind="ExternalInput")
    ident = nc.dram_tensor("ident", [P, P], MMD, kind="ExternalInput")
    y = nc.dram_tensor("y", [T, C], F32, kind="ExternalOutput")
    dbg = (nc.dram_tensor("dbg", [P, 3, 4224], MMD, kind="ExternalOutput")
           if phase != 'full' else None)

    with tile.TileContext(nc) as tc:
        with (
            tc.tile_pool(name="big", bufs=1) as big,
            tc.tile_pool(name="es_pool", bufs=4) as es_pool,
            tc.tile_pool(name="y_pool", bufs=3) as y_pool,
            tc.tile_pool(name="small", bufs=2) as small,
            tc.tile_pool(name="proj_ps", bufs=2, space="PSUM") as proj_ps,
            tc.tile_pool(name="sc_ps", bufs=2, space="PSUM") as sc_ps,
            tc.tile_pool(name="ctx_ps", bufs=1, space="PSUM") as ctx_ps,
        ):
            from contextlib import ExitStack
            _ls = ExitStack()
            if loop_n:
                _ls.enter_context(tc.For_i(0, loop_n, 1))
            xT_sb = big.tile([P, 8, T], MMD)
            wq_sb = big.tile([P, 8, CS], MMD)
            wk_sb = big.tile([P, 8, CS], MMD)
            wv_sb = big.tile([P, 8, CS], MMD)
            wp_sb = big.tile([P, 4, C], MMD)
            bq_sb = big.tile([P, 4], F32)
            mneg_sb = big.tile([P, P], MMD)
            id_sb = big.tile([P, P], MMD)
            qT_sb = big.tile([P, 4, 2, 512], MMD)
            kT_sb = big.tile([P, 4, 2, 512], MMD)
            v_sb = big.tile([P, 8, 8, VPAD], MMD)
            ctxT_sb = big.tile([P, 4, T], MMD)

            # Channel chunks use the "(p c)" split: partition p holds DRAM
            # rows p*8+c, i.e. 8 CONTIGUOUS rows -> one 8-16KB descriptor per
            # partition instead of eight 1-2KB ones.  The contraction is a
            # sum over all 1024 channels, and both matmul operands of every
            # chunk use the SAME permutation, so results are unchanged.
            nc.scalar.dma_start(out=bq_sb, in_=bq.ap())
            nc.scalar.dma_start(out=mneg_sb, in_=maskneg.ap())
            nc.scalar.dma_start(out=id_sb, in_=ident.ap())
            xT_r = xT.ap().rearrange("(p c) t -> p c t", p=P)
            wq_r = wq.ap().rearrange("(p c) n -> p c n", p=P)
            wk_r = wk.ap().rearrange("(p c) n -> p c n", p=P)
            wv_r = wv.ap().rearrange("(p c) n -> p c n", p=P)
            # xT+wv land first (V proj gates on them), one per HWDGE queue.
            nc.sync.dma_start(out=xT_sb, in_=xT_r)
            nc.scalar.dma_start(out=wv_sb, in_=wv_r)
            nc.sync.dma_start(out=wq_sb, in_=wq_r)
            nc.scalar.dma_start(out=wk_sb, in_=wk_r)
            wp_r = wp.ap().rearrange("(k p) n -> p k n", p=P)
            for kc in range(4):
                nc.sync.dma_start(out=wp_sb[:, kc, :], in_=wp_r[:, kc, :])

            # ---- V natural [T, 512] + ones column per head ----
            # Engine memset, NOT a scattered DMA: the stride-65 single-element
            # DMA write is not reliably ordered before the ctx matmul's first
            # read on HW (first-run garbage denominators).
            nc.vector.memset(v_sb[:, :, :, 64], 1.0)
            for tt in range(8):
                ps = proj_ps.tile([P, 512], F32, tag="proj")
                for c in range(8):
                    nc.tensor.matmul(
                        ps, xT_sb[:, c, tt * P:(tt + 1) * P], wv_sb[:, c, :],
                        start=(c == 0), stop=(c == 7))
                nc.vector.tensor_copy(
                    v_sb[:, tt, :, 0:64],
                    ps.rearrange("p (h d) -> p h d", h=8))

            _qk_done = set()

            def qk_proj_group(mc, gi):
                # gi: 0..3 = (wq,tc0),(wq,tc1),(wk,tc0),(wk,tc1)
                if (mc, gi) in _qk_done:
                    return
                _qk_done.add((mc, gi))
                wsb, outsb, is_q = ((wq_sb, qT_sb, True), (wk_sb, kT_sb, False))[gi // 2]
                tc2 = gi % 2
                ps = proj_ps.tile([P, 512], F32, tag="proj", name="qkps")
                for c in range(8):
                    nc.tensor.matmul(
                        ps, wsb[:, c, mc * P:(mc + 1) * P],
                        xT_sb[:, c, tc2 * 512:(tc2 + 1) * 512],
                        start=(c == 0), stop=(c == 7))
                dst = outsb[:, mc, tc2, :]
                if is_q:
                    nc.vector.tensor_scalar_add(dst, ps, bq_sb[:, mc:mc + 1])
                else:
                    nc.vector.tensor_copy(dst, ps)

            def qk_proj(mc):
                for gi in range(4):
                    qk_proj_group(mc, gi)

            def norm_write(h, qc, cps):
                hp = (h % 2) * 64
                mc = h // 2
                recr = small.tile([1, 512], F32, tag="recr", name="recr")
                nc.vector.reciprocal(recr, cps[64:65, :])
                recb = small.tile([64, 512], F32, tag="recb", name="recb")
                nc.gpsimd.partition_broadcast(recb, recr)
                nc.vector.tensor_mul(
                    ctxT_sb[hp:hp + 64, mc, qc * 512:(qc + 1) * 512],
                    cps[0:64, :], recb)

            _es_store = {}

            def emit_scores(h, kt):
                if (h, kt) in _es_store:
                    return
                hp = (h % 2) * 64
                mc = h // 2
                lhsT = kT_sb[hp:hp + 64, mc, kt // 4,
                             (kt % 4) * P:(kt % 4 + 1) * P]
                sps = sc_ps.tile([P, 2, 512], F32, tag="sc", name="sps")
                es = es_pool.tile([P, 2, 512], MMD, tag="es", name="es")
                _es_store[(h, kt)] = es
                r0 = (kt % 4) * P
                if kt < 4:
                    nc.tensor.matmul(sps[:, 0, r0:], lhsT,
                                     qT_sb[hp:hp + 64, mc, 0, r0:],
                                     start=True, stop=True)
                    nc.tensor.matmul(sps[:, 1, :], lhsT,
                                     qT_sb[hp:hp + 64, mc, 1, :],
                                     start=True, stop=True)
                else:
                    nc.tensor.matmul(sps[:, 0, r0:], lhsT,
                                     qT_sb[hp:hp + 64, mc, 1, r0:],
                                     start=True, stop=True)
                # causal mask: accumulate -1e4 above the diagonal, in PSUM
                nc.tensor.matmul(sps[:, 0, r0:r0 + P], id_sb, mneg_sb,
                                 start=False, stop=True,
                                 skip_group_check=True)
                if kt < 4:
                    if sexp:
                        nc.scalar.activation(es[:, 0, r0:], sps[:, 0, r0:],
                                             AF.Exp)
                        nc.scalar.activation(es[:, 1, :], sps[:, 1, :], AF.Exp)
                    else:
                        nc.scalar.activation(
                            es.rearrange("p a b -> p (a b)")[:, r0:],
                            sps.rearrange("p a b -> p (a b)")[:, r0:], AF.Exp)
                else:
                    nc.scalar.activation(es[:, 0, r0:], sps[:, 0, r0:], AF.Exp)

            def attention(h, pre_tail=None):
                # Software-pipelined over kt: ctx(kt-1) is emitted AFTER
                # scores(kt)+mask(kt), so the in-order PE stream never waits
                # for exp(kt) before it has scores(kt+1) to chew on.
                # pre_tail() emits independent PE work (next head's first
                # scores / next projection group) before the final ctx, which
                # is the one matmul that must wait for exp(7).
                hp = (h % 2) * 64
                cps0 = ctx_ps.tile([65, 512], F32, tag="ctx0", name="cps0")
                cps1 = ctx_ps.tile([65, 512], F32, tag="ctx1", name="cps1")

                def emit_ctx(kt):
                    es = _es_store.pop((h, kt))
                    r0 = (kt % 4) * P
                    v_l = v_sb[:, kt, h, 0:65]
                    if kt < 4:
                        nc.tensor.matmul(cps0[:, r0:], v_l, es[:, 0, r0:],
                                         start=(kt == 0), stop=(kt == 3))
                        nc.tensor.matmul(cps1, v_l, es[:, 1, :],
                                         start=(kt == 0), stop=(kt == 7))
                    else:
                        nc.tensor.matmul(cps1[:, r0:], v_l, es[:, 0, r0:],
                                         start=False, stop=(kt == 7))
                    if kt == 3:
                        norm_write(h, 0, cps0)

                emit_scores(h, 0)
                for kt in range(1, 8):
                    emit_scores(h, kt)
                    emit_ctx(kt - 1)
                if pre_tail is not None:
                    pre_tail()
                emit_ctx(7)
                norm_write(h, 1, cps1)

            def yproj(tt_range):
                for tt in tt_range:
                    ps = sc_ps.tile([P, 2, 512], F32, tag="sc", name="yps")
                    for kc in range(4):
                        lhsT = ctxT_sb[:, kc, tt * P:(tt + 1) * P]
                        nc.tensor.matmul(ps[:, 0, :], lhsT, wp_sb[:, kc, 0:512],
                                         start=(kc == 0), stop=(kc == 3))
                        nc.tensor.matmul(ps[:, 1, :], lhsT, wp_sb[:, kc, 512:],
                                         start=(kc == 0), stop=(kc == 3))
                    ysb = y_pool.tile([P, C], F32, tag="y", name="ysb")
                    nc.vector.tensor_copy(ysb, ps.rearrange("p a b -> p (a b)"))
                    (nc.sync, nc.scalar)[tt % 2].dma_start(
                        out=y.ap()[tt * P:(tt + 1) * P, :], in_=ysb)

            if phase == 'empty':
                pass
            elif phase == 'dma':
                for di, sb_t in enumerate((xT_sb, wq_sb, wk_sb, wv_sb, wp_sb)):
                    nch = sb_t.shape[1]
                    nc.sync.dma_start(
                        out=dbg.ap()[:, 0, di * 64:di * 64 + nch * 8],
                        in_=sb_t[:, :, :8])
            elif phase == 'proj':
                for mc in range(4):
                    qk_proj(mc)
                nc.sync.dma_start(out=dbg.ap()[:, 0, :4096],
                                  in_=qT_sb.rearrange("p a b c -> p (a b c)"))
                nc.sync.dma_start(out=dbg.ap()[:, 1, :4096],
                                  in_=kT_sb.rearrange("p a b c -> p (a b c)"))
                nc.sync.dma_start(out=dbg.ap()[:, 2, :4160],
                                  in_=v_sb[:, :, :, 0:65]
                                  .rearrange("p a b c -> p (a b c)"))
            elif phase == 'attn':
                for mc in range(4):
                    qk_proj(mc)
                    attention(2 * mc,
                              pre_tail=lambda m=mc: emit_scores(2 * m + 1, 0))
                    attention(2 * mc + 1,
                              pre_tail=(lambda m=mc: qk_proj_group(m + 1, 0))
                              if mc < 3 else None)
                nc.sync.dma_start(out=dbg.ap()[:, 0, :4096],
                                  in_=ctxT_sb.rearrange("p a b -> p (a b)"))
            else:
                for mc in range(4):
                    qk_proj(mc)
                    attention(2 * mc,
                              pre_tail=lambda m=mc: emit_scores(2 * m + 1, 0))
                    attention(2 * mc + 1,
                              pre_tail=(lambda m=mc: qk_proj_group(m + 1, 0))
                              if mc < 3 else None)
                    if mc == 3:
                        yproj(range(8))
            _ls.close()
    nc.compile()
    return nc


_NC = None


def _get_nc():
    global _NC
    if _NC is None:
        _NC = build_nc()
    return _NC


def make_in_maps(x, Wq, bq, Wk, Wv, Wp, mm_dtype=None):
    """Per-core input dicts."""
    import ml_dtypes
    MMD = mm_dtype or MM_DTYPE
    cvt = ((lambda a: np.ascontiguousarray(a).astype(ml_dtypes.bfloat16))
           if MMD == BF16 else np.ascontiguousarray)
    valid = (np.arange(P)[None, :] >= np.arange(P)[:, None])
    maskneg = np.where(valid, 0.0, -10000.0).astype(np.float32)
    in_maps = []
    for core in range(N_CORES):
        b = core // 2
        g = core % 2
        cs = slice(g * CS, (g + 1) * CS)
        in_maps.append(dict(
            xT=cvt(x[b].T),
            wq=cvt(Wq[:, cs] * np.float32(0.125)),
            wk=cvt(Wk[:, cs]),
            wv=cvt(Wv[:, cs]),
            wp=cvt(Wp[cs, :]),
            bq=np.ascontiguousarray((bq[cs] * np.float32(0.125))
                                    .reshape(4, P).T),
            maskneg=cvt(maskneg),
            ident=cvt(np.eye(P, dtype=np.float32)),
        ))
    return in_maps


def combine(parts, Wq, bv, Wp, bp):
    """parts: list of 8 per-core partial y arrays -> full [B, T, C] output."""
    out = np.stack([parts[2 * b] + parts[2 * b + 1] for b in range(B)])
    out += (bv @ Wp + bp)[None, None, :]
    return out.astype(np.float32)


def kernel(**inputs):
    x = np.asarray(inputs["x"], np.float32)
    Wq = np.asarray(inputs["Wq"], np.float32)
    bq = np.asarray(inputs["bq"], np.float32)
    Wk = np.asarray(inputs["Wk"], np.float32)
    Wv = np.asarray(inputs["Wv"], np.float32)
    Wp = np.asarray(inputs["Wp"], np.float32)
    bv = np.asarray(inputs["bv"], np.float32)
    bp = np.asarray(inputs["bp"], np.float32)
    # bk intentionally unused: it shifts every score of a query row by the
    # same amount, which softmax cancels exactly.

    nc = _get_nc()
    in_maps = make_in_maps(x, Wq, bq, Wk, Wv, Wp)
    res = run_bass_kernel_spmd(nc, in_maps, core_ids=list(range(N_CORES)))
    parts = [res.results[c]["y"] for c in range(N_CORES)]
    return combine(parts, Wq, bv, Wp, bp)
